# revision 9
# baseline (speedup 1.0000x reference)
import numpy as np
import ml_dtypes

# ---- problem constants (hardcoded from spec) ----
B, C, H, W = 2, 128, 256, 512
P = B * H * W               # 262144 pixels
TEMPERATURE = 0.1
BASE_TEMPERATURE = 0.07
MAX_SAMPLES = 1024
MAX_VIEWS = 100
NUM_CLASSES = 8
BIG_NEG = 1e9
N = NUM_CLASSES * MAX_SAMPLES   # 8192 sampled rows
N_CORES = 8
BLK = N // N_CORES              # 1024 rows/columns per core
SCALE = np.float32(BASE_TEMPERATURE / (TEMPERATURE * TEMPERATURE))  # 7.0f exactly

_PROGRAM = {}


def _sample_indices_host(labels_flat_np):
    """Verbatim replication of reference._sample_indices on jax-CPU."""
    import jax
    import jax.numpy as jnp

    cpu = jax.devices("cpu")[0]
    with jax.default_device(cpu):
        labels_flat = jnp.asarray(labels_flat_np)
        key = jax.random.key(42)
        k1, k2 = jax.random.split(key)
        scores = jax.random.uniform(k1, (P,))
        class_mask = (
            labels_flat[None, :]
            == jnp.arange(NUM_CLASSES, dtype=labels_flat.dtype)[:, None]
        )
        masked_scores = jnp.where(class_mask, scores[None, :], -1.0)
        _, idx = jax.lax.top_k(masked_scores, MAX_SAMPLES)
        sampled_idx = idx.reshape(-1)
        row_scores = jax.random.uniform(k2, (N, MAX_SAMPLES))
        _, sel = jax.lax.top_k(row_scores, MAX_VIEWS)
        block_start = (jnp.arange(N) // MAX_SAMPLES) * MAX_SAMPLES
        pos_cols = sel + block_start[:, None]
        return np.asarray(sampled_idx), np.asarray(pos_cols)


NK = 5                  # cyclic block-columns computed per core (k = 0..4)
KC = NK * BLK           # 5120 columns of embR actually needed per core


def _build_program():
    """Bass/Tile SPMD program (shared by all 8 cores).

    Symmetry scheme: exp(7*G) is symmetric; each core computes its 1024-row
    block against cyclic column blocks k=0..4 (5/8 of the matrix).  The
    diagonal is NOT suppressed on device — the host subtracts exp(7*g_jj)
    (replicating the bf16 quantization) from the combined column sums.

    Per chunk c (128 rows), three PSUM tiles of 2048 columns each are
    matmul'd and exp'd by ACT into a persistent SBUF e-arena laid out in
    three sections: [k0k1 | k2k3 | k4].  k4 tiles of two adjacent chunks
    share one 2048-wide ACT.  A DVE scalar_tensor_tensor (4x bf16 mode)
    accumulates e into csacc per section and emits running row-sum
    accumulators; the host recovers per-chunk row sums by telescoping
    differences.  Column sums of the k1..k3 sections (csacc[:,1024:4096])
    are partition-summed on the host."""
    if _PROGRAM:
        return _PROGRAM

    import concourse.mybir as mybir
    from concourse import bacc, tile

    f32 = mybir.dt.float32
    bf16 = mybir.dt.bfloat16
    Alu = mybir.AluOpType

    nc = bacc.Bacc("TRN2", target_bir_lowering=False)

    # embR: row-normalized embeddings, transposed [C, N], rolled so this
    # core's own 1024-column class block sits at columns 0..1023.
    embR_d = nc.dram_tensor("embR", [128, KC], bf16, kind="ExternalInput")
    cs_d = nc.dram_tensor("cs", [128, 3 * BLK], bf16, kind="ExternalOutput")
    accs_d = nc.dram_tensor("accs", [128, 24], f32, kind="ExternalOutput")

    with tile.TileContext(nc) as tc:
        with (
            tc.tile_pool(name="persist", bufs=1) as persist,
            tc.tile_pool(name="psum", bufs=2, space="PSUM") as psum,
        ):
            embR = persist.tile([128, KC], bf16)
            earena = persist.tile([128, 8 * KC], bf16)   # 80KB/partition
            csacc = persist.tile([128, 3 * BLK], bf16)   # [k1 | k2 | k3]
            accs = persist.tile([128, 24], f32)          # [k0k1 | k2k3 | k4]
            scratch = persist.tile([128, 2048], bf16)    # rowsum dummy out

            # stream embR in; first cut unblocks the first matmul quickly
            emb_cuts = [(0, 512), (512, 1024), (1024, 2048),
                        (2048, 3072), (3072, 4096), (4096, KC)]
            for lo, hi in emb_cuts:
                nc.sync.dma_start(out=embR[:, lo:hi], in_=embR_d[:, lo:hi])

            # section base offsets in earena column space
            S_K01 = 0            # 2048 per chunk  (cols 0:2048 of embR)
            S_K23 = 8 * 2048     # 2048 per chunk  (cols 2048:4096)
            S_K4 = 8 * 4096      # 1024 per chunk  (cols 4096:5120)

            def mm_tile(ps, c, col0):
                """4 matmuls filling ps[128,2048] = rows of chunk c x embR
                cols [col0, col0+2048)."""
                lhsT = embR[:, c * 128:(c + 1) * 128]
                for t in range(4):
                    nc.tensor.matmul(
                        ps[:, t * 512:(t + 1) * 512],
                        lhsT,
                        embR[:, col0 + t * 512: col0 + (t + 1) * 512],
                        start=True, stop=True,
                    )

            # zero csacc up front (DVE is idle during the DMA prologue)
            nc.vector.memset(csacc[:], 0.0)

            def rowsum(e_ap, acc_col):
                """Identity tensor_scalar (4x bf16) into a scratch tile;
                accum_out is the per-partition row sum of e_ap."""
                w = e_ap.shape[1]
                nc.vector.tensor_scalar(
                    out=scratch[:, 0:w], in0=e_ap, scalar1=0.0, scalar2=0.0,
                    op0=Alu.add, op1=Alu.add,
                    accum_out=accs[:, acc_col:acc_col + 1],
                )

            def csadd(lo, width, e_ap):
                """csacc[lo:lo+width] += e (bf16 tensor_tensor, 2x)."""
                cs_ap = csacc[:, lo:lo + width]
                nc.vector.tensor_tensor(
                    out=cs_ap, in0=cs_ap, in1=e_ap, op=Alu.add,
                )

            def act_exp(e_ap, ps):
                nc.scalar.activation(
                    e_ap, ps[:], mybir.ActivationFunctionType.Exp,
                    scale=float(SCALE),
                )

            for pair in range(4):
                c0, c1 = 2 * pair, 2 * pair + 1
                # k0k1 + k2k3 of c0, k0k1 of c1, then k4 pair, then k2k3 of
                # c1 last — keeps the kernel tail short (only the final
                # k2k3 csadd + cs DMA follow the last ACT).
                ps = psum.tile([128, 2048], f32, tag="ps")
                mm_tile(ps, c0, 0)
                e01 = earena[:, S_K01 + c0 * 2048: S_K01 + (c0 + 1) * 2048]
                act_exp(e01, ps)
                csadd(0, 1024, e01[:, 1024:2048])
                rowsum(e01, c0)

                ps = psum.tile([128, 2048], f32, tag="ps")
                mm_tile(ps, c0, 2048)
                e23 = earena[:, S_K23 + c0 * 2048: S_K23 + (c0 + 1) * 2048]
                act_exp(e23, ps)
                csadd(1024, 2048, e23)
                rowsum(e23, 8 + c0)

                ps = psum.tile([128, 2048], f32, tag="ps")
                mm_tile(ps, c1, 0)
                e01 = earena[:, S_K01 + c1 * 2048: S_K01 + (c1 + 1) * 2048]
                act_exp(e01, ps)
                csadd(0, 1024, e01[:, 1024:2048])
                rowsum(e01, c1)
                if c1 == 7:
                    # k1 column sums are final; stream out early
                    nc.sync.dma_start(out=cs_d[:, 0:1024], in_=csacc[:, 0:1024])

                # k4 for both chunks of the pair, one 2048-wide ACT
                ps = psum.tile([128, 2048], f32, tag="ps")
                for i, c in enumerate((c0, c1)):
                    lhsT = embR[:, c * 128:(c + 1) * 128]
                    for t in range(2):
                        nc.tensor.matmul(
                            ps[:, i * 1024 + t * 512: i * 1024 + (t + 1) * 512],
                            lhsT,
                            embR[:, 4096 + t * 512: 4096 + (t + 1) * 512],
                            start=True, stop=True,
                        )
                e4 = earena[:, S_K4 + c0 * 1024: S_K4 + (c1 + 1) * 1024]
                act_exp(e4, ps)
                rowsum(e4[:, 0:1024], 16 + c0)
                rowsum(e4[:, 1024:2048], 16 + c1)

                ps = psum.tile([128, 2048], f32, tag="ps")
                mm_tile(ps, c1, 2048)
                e23 = earena[:, S_K23 + c1 * 2048: S_K23 + (c1 + 1) * 2048]
                act_exp(e23, ps)
                csadd(1024, 2048, e23)
                rowsum(e23, 8 + c1)

            nc.sync.dma_start(out=cs_d[:, 1024:3072], in_=csacc[:, 1024:3072])
            nc.sync.dma_start(out=accs_d[:], in_=accs[:])

    nc.finalize()
    _PROGRAM["nc"] = nc
    return _PROGRAM


def _spos_host(emb_n, pos_cols):
    """s_pos = sum of exp(7*dot) over all (row, pos) pairs, excluding
    self-pairs (suppressed to exactly 0 in the reference)."""
    rows = np.repeat(np.arange(N), MAX_VIEWS)
    cols = pos_cols.ravel()
    mask = cols != rows
    rows, cols = rows[mask], cols[mask]
    total = 0.0
    for ofs in range(0, rows.size, 131072):
        r = rows[ofs:ofs + 131072]
        c = cols[ofs:ofs + 131072]
        dots = np.einsum("ij,ij->i", emb_n[r], emb_n[c], dtype=np.float64)
        total += float(np.exp(np.float64(SCALE) * dots).sum())
    return total


def _host_prep(embeddings, labels):
    sampled_idx, pos_cols = _sample_indices_host(labels.reshape(-1))
    hw = H * W
    b = sampled_idx // hw
    h = (sampled_idx % hw) // W
    w = sampled_idx % W
    emb_s = embeddings[b, :, h, w].astype(np.float32)  # [N, C]
    norm = np.sqrt(np.sum(emb_s * emb_s, axis=1, dtype=np.float32)).astype(np.float32)
    norm = np.maximum(norm, np.float32(1e-12))
    emb_n = emb_s / norm[:, None]
    embT = np.ascontiguousarray(emb_n.T).astype(ml_dtypes.bfloat16)  # [C, N]

    spos = _spos_host(emb_n, pos_cols)

    # diagonal correction: exp(7 * g_jj) with the same bf16 quantization the
    # device matmul sees
    q = embT.astype(np.float64)
    diag_e = np.exp(np.float64(SCALE) * (q * q).sum(axis=0))  # [N]

    in_maps = []
    for m in range(N_CORES):
        embR = np.ascontiguousarray(np.roll(embT, -BLK * m, axis=1)[:, :KC])
        in_maps.append({"embR": embR})
    return in_maps, (spos, diag_e)


def _combine(results, host_data):
    spos, diag_e = host_data
    rowsums, cs_k = [], []
    for res in results:
        accs = np.asarray(res["accs"], dtype=np.float64)  # [128, 24]
        rs = accs[:, 0:8] + accs[:, 8:16] + accs[:, 16:24]  # [128, 8]
        rowsums.append(rs.T.reshape(-1))  # [1024], u = c*128 + p
        cs_k.append(np.asarray(res["cs"], dtype=np.float64).sum(axis=0))  # [3072]
    col_sum = np.empty(N, dtype=np.float64)
    for bblk in range(N_CORES):
        col_sum[bblk * BLK:(bblk + 1) * BLK] = (
            rowsums[bblk]
            + cs_k[(bblk - 1) % N_CORES][0:1024]
            + cs_k[(bblk - 2) % N_CORES][1024:2048]
            + cs_k[(bblk - 3) % N_CORES][2048:3072]
            - diag_e[bblk * BLK:(bblk + 1) * BLK]
        )
    loss = -np.log(spos) + np.mean(np.log(col_sum))
    return np.float32(loss)


def kernel(embeddings: np.ndarray, labels: np.ndarray) -> np.ndarray:
    from concourse.bass_utils import run_bass_kernel_spmd

    prog = _build_program()
    in_maps, host_data = _host_prep(np.asarray(embeddings), np.asarray(labels))
    out = run_bass_kernel_spmd(prog["nc"], in_maps, list(range(N_CORES)))
    return _combine(out.results, host_data)


# revision 13
# speedup vs baseline: 1.3185x; 1.3185x over previous
import numpy as np
import ml_dtypes

# ---- problem constants (hardcoded from spec) ----
B, C, H, W = 2, 128, 256, 512
P = B * H * W               # 262144 pixels
TEMPERATURE = 0.1
BASE_TEMPERATURE = 0.07
MAX_SAMPLES = 1024
MAX_VIEWS = 100
NUM_CLASSES = 8
BIG_NEG = 1e9
N = NUM_CLASSES * MAX_SAMPLES   # 8192 sampled rows
N_CORES = 8
BLK = N // N_CORES              # 1024 rows/columns per core
SCALE = np.float32(BASE_TEMPERATURE / (TEMPERATURE * TEMPERATURE))  # 7.0f exactly

_PROGRAM = {}


def _sample_indices_host(labels_flat_np):
    """Verbatim replication of reference._sample_indices on jax-CPU."""
    import jax
    import jax.numpy as jnp

    cpu = jax.devices("cpu")[0]
    with jax.default_device(cpu):
        labels_flat = jnp.asarray(labels_flat_np)
        key = jax.random.key(42)
        k1, k2 = jax.random.split(key)
        scores = jax.random.uniform(k1, (P,))
        class_mask = (
            labels_flat[None, :]
            == jnp.arange(NUM_CLASSES, dtype=labels_flat.dtype)[:, None]
        )
        masked_scores = jnp.where(class_mask, scores[None, :], -1.0)
        _, idx = jax.lax.top_k(masked_scores, MAX_SAMPLES)
        sampled_idx = idx.reshape(-1)
        row_scores = jax.random.uniform(k2, (N, MAX_SAMPLES))
        _, sel = jax.lax.top_k(row_scores, MAX_VIEWS)
        block_start = (jnp.arange(N) // MAX_SAMPLES) * MAX_SAMPLES
        pos_cols = sel + block_start[:, None]
        return np.asarray(sampled_idx), np.asarray(pos_cols)


NK = 5                  # cyclic block-columns computed per core (k = 0..4)
KC = NK * BLK           # 5120 columns of embR actually needed per core


def _build_program():
    """Bass/Tile SPMD program (shared by all 8 cores).

    Symmetry scheme: exp(7*G) is symmetric; each core computes its 1024-row
    block against cyclic column blocks k=0..4 (5/8 of the matrix).  The
    diagonal is NOT suppressed on device — the host subtracts exp(7*g_jj)
    (replicating the bf16 quantization) from the combined column sums.

    Per chunk c (128 rows), three PSUM tiles of 2048 columns each are
    matmul'd and exp'd by ACT into a persistent SBUF e-arena laid out in
    three sections: [k0k1 | k2k3 | k4].  k4 tiles of two adjacent chunks
    share one 2048-wide ACT.  A DVE scalar_tensor_tensor (4x bf16 mode)
    accumulates e into csacc per section and emits running row-sum
    accumulators; the host recovers per-chunk row sums by telescoping
    differences.  Column sums of the k1..k3 sections (csacc[:,1024:4096])
    are partition-summed on the host."""
    if _PROGRAM:
        return _PROGRAM

    import concourse.mybir as mybir
    from concourse import bacc, tile

    f32 = mybir.dt.float32
    bf16 = mybir.dt.bfloat16
    Alu = mybir.AluOpType

    nc = bacc.Bacc("TRN2", target_bir_lowering=False)

    # embR: row-normalized embeddings, transposed [C, N], rolled so this
    # core's own 1024-column class block sits at columns 0..1023.
    embR_d = nc.dram_tensor("embR", [128, KC], bf16, kind="ExternalInput")
    cs_d = nc.dram_tensor("cs", [128, 4 * BLK], bf16, kind="ExternalOutput")
    accs_d = nc.dram_tensor("accs", [128, 16], f32, kind="ExternalOutput")

    with tile.TileContext(nc) as tc:
        with (
            tc.tile_pool(name="persist", bufs=1) as persist,
            tc.tile_pool(name="psum", bufs=2, space="PSUM") as psum,
        ):
            embR = persist.tile([128, KC], bf16)
            earena = persist.tile([128, 8 * KC], bf16)   # 80KB/partition
            csacc = persist.tile([128, 4 * BLK], bf16)   # [k1 | k2 | k3 | k4]
            accs = persist.tile([128, 16], f32)          # [k0k1 | k2k3] rowsums

            # stream embR in; first cut unblocks the first matmul quickly
            emb_cuts = [(0, 512), (512, 1024), (1024, 2048),
                        (2048, 3072), (3072, 4096), (4096, KC)]
            for lo, hi in emb_cuts:
                nc.sync.dma_start(out=embR[:, lo:hi], in_=embR_d[:, lo:hi])

            # section base offsets in earena column space
            S_K01 = 0            # 2048 per chunk  (cols 0:2048 of embR)
            S_K23 = 8 * 2048     # 2048 per chunk  (cols 2048:4096)
            S_K4 = 8 * 4096      # 1024 per chunk  (cols 4096:5120)

            def mm_tile(ps, c, col0):
                """4 matmuls filling ps[128,2048] = rows of chunk c x embR
                cols [col0, col0+2048)."""
                lhsT = embR[:, c * 128:(c + 1) * 128]
                for t in range(4):
                    nc.tensor.matmul(
                        ps[:, t * 512:(t + 1) * 512],
                        lhsT,
                        embR[:, col0 + t * 512: col0 + (t + 1) * 512],
                        start=True, stop=True,
                    )

            # zero csacc up front (DVE is idle during the DMA prologue)
            nc.vector.memset(csacc[:], 0.0)

            def csadd(lo, width, e_ap):
                """csacc[lo:lo+width] += e (bf16 tensor_tensor, 2x)."""
                cs_ap = csacc[:, lo:lo + width]
                nc.vector.tensor_tensor(
                    out=cs_ap, in0=cs_ap, in1=e_ap, op=Alu.add,
                )

            def act_exp(e_ap, ps, acc_col=None):
                acc = None if acc_col is None else accs[:, acc_col:acc_col + 1]
                nc.scalar.activation(
                    e_ap, ps[:], mybir.ActivationFunctionType.Exp,
                    scale=float(SCALE), accum_out=acc,
                )

            for pair in range(4):
                c0, c1 = 2 * pair, 2 * pair + 1
                # k0k1 + k2k3 of c0, k0k1 of c1, then k4 pair, then k2k3 of
                # c1 last — keeps the kernel tail short (only the final
                # k2k3 csadd + cs DMA follow the last ACT).
                ps = psum.tile([128, 2048], f32, tag="ps")
                mm_tile(ps, c0, 0)
                e01 = earena[:, S_K01 + c0 * 2048: S_K01 + (c0 + 1) * 2048]
                act_exp(e01, ps, c0)
                csadd(0, 1024, e01[:, 1024:2048])

                ps = psum.tile([128, 2048], f32, tag="ps")
                mm_tile(ps, c0, 2048)
                e23 = earena[:, S_K23 + c0 * 2048: S_K23 + (c0 + 1) * 2048]
                act_exp(e23, ps, 8 + c0)
                csadd(1024, 2048, e23)

                ps = psum.tile([128, 2048], f32, tag="ps")
                mm_tile(ps, c1, 0)
                e01 = earena[:, S_K01 + c1 * 2048: S_K01 + (c1 + 1) * 2048]
                act_exp(e01, ps, c1)
                csadd(0, 1024, e01[:, 1024:2048])
                if c1 == 7:
                    # k1 column sums are final; stream out early
                    nc.sync.dma_start(out=cs_d[:, 0:1024], in_=csacc[:, 0:1024])

                # k4 for both chunks of the pair, one 2048-wide ACT (rows of
                # two different chunks -> no accum; k4 rowsums come from the
                # partner core's csacc k4 section by symmetry)
                ps = psum.tile([128, 2048], f32, tag="ps")
                for i, c in enumerate((c0, c1)):
                    lhsT = embR[:, c * 128:(c + 1) * 128]
                    for t in range(2):
                        nc.tensor.matmul(
                            ps[:, i * 1024 + t * 512: i * 1024 + (t + 1) * 512],
                            lhsT,
                            embR[:, 4096 + t * 512: 4096 + (t + 1) * 512],
                            start=True, stop=True,
                        )
                e4 = earena[:, S_K4 + c0 * 1024: S_K4 + (c1 + 1) * 1024]
                act_exp(e4, ps)
                csadd(3072, 1024, e4[:, 0:1024])
                csadd(3072, 1024, e4[:, 1024:2048])
                if c1 == 7:
                    nc.sync.dma_start(out=cs_d[:, 3072:4096],
                                      in_=csacc[:, 3072:4096])

                ps = psum.tile([128, 2048], f32, tag="ps")
                mm_tile(ps, c1, 2048)
                e23 = earena[:, S_K23 + c1 * 2048: S_K23 + (c1 + 1) * 2048]
                act_exp(e23, ps, 8 + c1)
                csadd(1024, 2048, e23)

            nc.sync.dma_start(out=cs_d[:, 1024:3072], in_=csacc[:, 1024:3072])
            nc.sync.dma_start(out=accs_d[:], in_=accs[:])

    nc.finalize()
    _PROGRAM["nc"] = nc
    return _PROGRAM


def _spos_host(emb_n, pos_cols):
    """s_pos = sum of exp(7*dot) over all (row, pos) pairs, excluding
    self-pairs (suppressed to exactly 0 in the reference)."""
    rows = np.repeat(np.arange(N), MAX_VIEWS)
    cols = pos_cols.ravel()
    mask = cols != rows
    rows, cols = rows[mask], cols[mask]
    total = 0.0
    for ofs in range(0, rows.size, 131072):
        r = rows[ofs:ofs + 131072]
        c = cols[ofs:ofs + 131072]
        dots = np.einsum("ij,ij->i", emb_n[r], emb_n[c], dtype=np.float64)
        total += float(np.exp(np.float64(SCALE) * dots).sum())
    return total


def _host_prep(embeddings, labels):
    sampled_idx, pos_cols = _sample_indices_host(labels.reshape(-1))
    hw = H * W
    b = sampled_idx // hw
    h = (sampled_idx % hw) // W
    w = sampled_idx % W
    emb_s = embeddings[b, :, h, w].astype(np.float32)  # [N, C]
    norm = np.sqrt(np.sum(emb_s * emb_s, axis=1, dtype=np.float32)).astype(np.float32)
    norm = np.maximum(norm, np.float32(1e-12))
    emb_n = emb_s / norm[:, None]
    embT = np.ascontiguousarray(emb_n.T).astype(ml_dtypes.bfloat16)  # [C, N]

    spos = _spos_host(emb_n, pos_cols)

    # diagonal correction: exp(7 * g_jj) with the same bf16 quantization the
    # device matmul sees
    q = embT.astype(np.float64)
    diag_e = np.exp(np.float64(SCALE) * (q * q).sum(axis=0))  # [N]

    in_maps = []
    for m in range(N_CORES):
        embR = np.ascontiguousarray(np.roll(embT, -BLK * m, axis=1)[:, :KC])
        in_maps.append({"embR": embR})
    return in_maps, (spos, diag_e)


def _combine(results, host_data):
    spos, diag_e = host_data
    rowsums, cs_k = [], []
    for res in results:
        accs = np.asarray(res["accs"], dtype=np.float64)  # [128, 16]
        rs = accs[:, 0:8] + accs[:, 8:16]  # rowsum over k0..k3, [128, 8]
        rowsums.append(rs.T.reshape(-1))  # [1024], u = c*128 + p
        cs_k.append(np.asarray(res["cs"], dtype=np.float64).sum(axis=0))  # [4096]
    col_sum = np.empty(N, dtype=np.float64)
    for bblk in range(N_CORES):
        col_sum[bblk * BLK:(bblk + 1) * BLK] = (
            rowsums[bblk]
            + cs_k[(bblk - 1) % N_CORES][0:1024]
            + cs_k[(bblk - 2) % N_CORES][1024:2048]
            + cs_k[(bblk - 3) % N_CORES][2048:3072]
            + cs_k[(bblk + 4) % N_CORES][3072:4096]
            - diag_e[bblk * BLK:(bblk + 1) * BLK]
        )
    loss = -np.log(spos) + np.mean(np.log(col_sum))
    return np.float32(loss)


def kernel(embeddings: np.ndarray, labels: np.ndarray) -> np.ndarray:
    from concourse.bass_utils import run_bass_kernel_spmd

    prog = _build_program()
    in_maps, host_data = _host_prep(np.asarray(embeddings), np.asarray(labels))
    out = run_bass_kernel_spmd(prog["nc"], in_maps, list(range(N_CORES)))
    return _combine(out.results, host_data)
